# revision 9
# baseline (speedup 1.0000x reference)
"""DeformableConvV2 (DCNv2) Trainium2 Bass kernel, v2.

Problem (hardcoded): x [4,256,48,48] f32, w_offset [27,256,3,3], w_dcn
[256,256,3,3]; stride 1, dil 1, same padding -> out [4,256,48,48] f32.

Strategy: 8 cores, each handles half a sample (24 output rows, p=1152
positions).  Per core:
  1. offset/mask conv on PE (bf16 operands, f32 PSUM), pipelined per
     384-position n-tile so index math starts early
  2. transpose om to [p, 27]; merged y/x fp32 index math on DVE over
     interleaved [p, 18] tiles; bilinear+mask coeffs packed to
     apack [p, k, 4] bf16
  3. int16 element indices -> dma_gather from a host-prepared "dup-row"
     NHWC bf16 image: one 2KB element = all 4 bilinear neighbors (256ch)
  4. per p-chunk: diag tiles built in bulk (one DVE tensor_tensor with
     broadcast APs for neighbors 0-2, ACT muls for neighbor 3), then
     fused scale+transpose on PE: psum[c, q] += gt_n^T @ diag(a_n)
  5. main contraction on PE from patches [(k,c), p]
"""
import numpy as np
import ml_dtypes
from contextlib import ExitStack

import concourse.bass as bass
import concourse.bacc as bacc
import concourse.mybir as mybir
from concourse.tile import TileContext

bf16 = ml_dtypes.bfloat16
F32 = mybir.dt.float32
BF16 = mybir.dt.bfloat16
I16 = mybir.dt.int16
I32 = mybir.dt.int32
ALU = mybir.AluOpType
ACTF = mybir.ActivationFunctionType

B, CIN, COUT, K, H, W = 4, 256, 256, 3, 48, 48
K2 = K * K
NCORES = 8
ROWS = H // 2              # output rows per core = 24
P = ROWS * W               # positions per core = 1152
PC = P // 128              # p-chunks per core = 9
CC = 2 * K2                # contraction chunks = 18  (k*2 + c_half)
XROWS = ROWS + 2           # padded x rows needed for om conv = 26
X2N = 50 * 50 + 8          # dup-row gather source rows (+slack)
NTAP = 3                   # main-matmul n-tiles of 384 positions


def build_nc(stage=99):
    """Build the single SPMD program (same for all 8 cores).

    stage: debug truncation level (99 = full kernel):
      2 = loads + om conv + index math (slotf out);  3 = + gathers;
      4 = + patches;  99 = full
    """
    nc = bacc.Bacc(num_swdge_queues=2)

    xc_d = nc.declare_dram_parameter("xcyx", [2, 128, XROWS * 50], BF16, isOutput=False)
    x2_d = nc.declare_dram_parameter("x2", [X2N * 512], BF16, isOutput=False)
    byx_d = nc.declare_dram_parameter("baseyx", [128, PC, 18], F32, isOutput=False)
    wo_d = nc.declare_dram_parameter("woff", [128, CC, 27], BF16, isOutput=False)
    wd_d = nc.declare_dram_parameter("wdcn", [128, CC, 256], BF16, isOutput=False)
    idf_d = nc.declare_dram_parameter("identf", [128, 128], F32, isOutput=False)
    idb_d = nc.declare_dram_parameter("identb", [128, 128], BF16, isOutput=False)
    out_d = nc.declare_dram_parameter("out", [2, 128, P], F32, isOutput=True)

    with TileContext(nc) as tc, ExitStack() as ctx:
        const = ctx.enter_context(tc.tile_pool(name="const", bufs=1))
        work = ctx.enter_context(tc.tile_pool(name="work", bufs=1))
        gpool = ctx.enter_context(tc.tile_pool(name="gpool", bufs=3))
        dgpool = ctx.enter_context(tc.tile_pool(name="dgpool", bufs=3))
        ps_om = ctx.enter_context(tc.tile_pool(name="ps_om", bufs=1, space="PSUM"))
        ps_tr = ctx.enter_context(tc.tile_pool(name="ps_tr", bufs=2, space="PSUM"))
        ps_mm = ctx.enter_context(tc.tile_pool(name="ps_mm", bufs=2, space="PSUM"))

        # ---------------- loads (om-conv critical ones first) ----------------
        wof = const.tile([128, CC, 27], BF16)
        nc.sync.dma_start(out=wof[:], in_=wo_d[:])
        xc = const.tile([128, 2, XROWS * 50], BF16)
        nc.sync.dma_start(out=xc[:], in_=xc_d.rearrange("a p f -> p a f"))
        identf = const.tile([128, 128], F32)
        nc.scalar.dma_start(out=identf[:], in_=idf_d[:])
        baseyx = const.tile([128, PC, 18], F32)
        nc.scalar.dma_start(out=baseyx[:], in_=byx_d[:])
        identb = const.tile([128, 128], BF16)
        nc.sync.dma_start(out=identb[:], in_=idb_d[:])
        wdc = const.tile([128, CC, 256], BF16)
        nc.scalar.dma_start(out=wdc[:], in_=wd_d[:])

        # persistent tiles
        om_sb = work.tile([27, P], F32)
        omt = work.tile([128, PC, 27], F32)
        msk = work.tile([128, PC, K2], F32)
        apack = work.tile([128, PC, K2, 3], BF16)
        a3 = work.tile([128, PC, K2], F32)
        slot16 = work.tile([128, PC, K2], I16)
        idxtmp = work.tile([128, 8, PC, K2], I16)
        idxbuf = work.tile([128, PC, K2, 8], I16)
        patches = work.tile([128, CC, P], BF16)
        out_sb = work.tile([128, 2, P], F32)

        shape18 = [128, PC, 18]

        def wt18(name, dt=F32):
            return work.tile(shape18, dt, name=name)

        def wt9(name, dt=F32):
            return work.tile([128, PC, K2], dt, name=name)

        t_ = wt18("t_")
        ti = wt18("ti", I32)
        tf = wt18("tf")
        fx = wt18("fx")
        fl = wt18("fl")
        wv = wt18("wv")
        v0 = wt18("v0")
        ov = wt18("ov")
        ulo = wt18("ulo")
        alo = wt18("alo")
        uhi = wt18("uhi")
        ahi = wt18("ahi")
        am0 = wt9("am0")
        am1 = wt9("am1")
        yc = wt9("yc")
        xcl = wt9("xcl")
        t1 = wt9("t1")
        slotf = wt9("slotf")

        # ---------------- om conv for one n-tile ----------------
        def emit_om(nt):
            ps = ps_om.tile([27, 384], F32, tag="om")
            first = True
            for k in range(K2):
                ki, kj = k // K, k % K
                r0 = nt * 8 + ki
                for ch in range(2):
                    rhs = xc[:, ch].rearrange("p (r j) -> p r j", r=XROWS)[
                        :, r0:r0 + 8, kj:kj + 48]
                    nc.tensor.matmul(
                        ps[:], wof[:, k * 2 + ch, :], rhs,
                        start=first, stop=(k == K2 - 1 and ch == 1))
                    first = False
            nc.scalar.copy(om_sb[:, nt * 384:(nt + 1) * 384], ps[:])

        # ---------------- omt + gather-index math for a group of pcs ----------
        def emit_idx(pcs):
            sl = slice(pcs[0], pcs[-1] + 1)
            for pc in pcs:
                pst = ps_tr.tile([128, 27], F32, tag="omt")
                nc.tensor.transpose(pst[:], om_sb[:, pc * 128:(pc + 1) * 128],
                                    identf[0:27, 0:27])
                nc.vector.tensor_copy(omt[:, pc, :], pst[:])

            d_all = omt[:, sl, 0:18]
            # floor via +64, int cast, and a fix for round-up casts:
            #   t = d + 64;  ti = cast_int(t);  fi = ti - (ti > t)  == floor(t)
            nc.vector.tensor_scalar_add(t_[:, sl], d_all, 64.0)
            nc.vector.tensor_copy(ti[:, sl], t_[:, sl])
            nc.vector.tensor_copy(tf[:, sl], ti[:, sl])
            nc.vector.tensor_tensor(fx[:, sl], tf[:, sl], t_[:, sl], ALU.is_gt)
            nc.vector.tensor_sub(fl[:, sl], tf[:, sl], fx[:, sl])
            nc.vector.tensor_sub(wv[:, sl], t_[:, sl], fl[:, sl])
            nc.vector.tensor_add(v0[:, sl], fl[:, sl], baseyx[:, sl])

            # int element index into the dup-row table:
            #   slot = (yc+1)*50 + (xc+1), yc in [-1,47], xc in [-1,48]
            nc.vector.tensor_scalar(yc[:, sl], v0[:, sl, 0:18:2], -1.0, 47.0,
                                    ALU.max, ALU.min)
            nc.vector.tensor_scalar(xcl[:, sl], v0[:, sl, 1:18:2], -1.0, 48.0,
                                    ALU.max, ALU.min)
            nc.vector.tensor_scalar(t1[:, sl], yc[:, sl], 50.0, 51.0,
                                    ALU.mult, ALU.add)
            nc.vector.tensor_add(slotf[:, sl], t1[:, sl], xcl[:, sl])
            nc.vector.tensor_copy(slot16[:, sl], slotf[:, sl])

            # idx fold: [128, pc, k] -> 16-partition wrap + hi shuffle + replicate
            for hi in range(8):
                eng = nc.sync if hi % 2 == 0 else nc.scalar
                eng.dma_start(out=idxtmp[0:16, hi, sl, :],
                              in_=slot16[hi * 16:(hi + 1) * 16, sl, :])
            nc.vector.tensor_copy(
                idxbuf[0:16, sl, :, :],
                idxtmp[0:16, :, sl, :].rearrange("p a b c -> p b c a"))
            for r in range(7):   # 7 parallel replicas of partitions 0:16
                eng = nc.sync if r % 2 == 0 else nc.scalar
                eng.dma_start(out=idxbuf[(r + 1) * 16:(r + 2) * 16, sl],
                              in_=idxbuf[0:16, sl])

            # mask (ACT)
            nc.scalar.activation(msk[:, sl], omt[:, sl, 18:27], ACTF.Sigmoid)

        def emit_coef(pcs):
            sl = slice(pcs[0], pcs[-1] + 1)
            # bilinear weights x validity, y and x interleaved:
            #   alo = (1-w) * [v0 in [0,47]];  ahi = w * [v0+1 in [0,47]]
            nc.vector.tensor_scalar(ov[:, sl], wv[:, sl], -1.0, 1.0,
                                    ALU.mult, ALU.add)
            nc.vector.scalar_tensor_tensor(ulo[:, sl], v0[:, sl], 47.5,
                                           ov[:, sl], ALU.is_le, ALU.mult)
            nc.vector.scalar_tensor_tensor(alo[:, sl], v0[:, sl], -0.5,
                                           ulo[:, sl], ALU.is_ge, ALU.mult)
            nc.vector.scalar_tensor_tensor(uhi[:, sl], v0[:, sl], 46.5,
                                           wv[:, sl], ALU.is_le, ALU.mult)
            nc.vector.scalar_tensor_tensor(ahi[:, sl], v0[:, sl], -1.5,
                                           uhi[:, sl], ALU.is_ge, ALU.mult)

            nc.vector.tensor_mul(am0[:, sl], alo[:, sl, 0:18:2], msk[:, sl])
            nc.vector.tensor_mul(am1[:, sl], ahi[:, sl, 0:18:2], msk[:, sl])
            nc.vector.tensor_mul(apack[:, sl, :, 0], am0[:, sl], alo[:, sl, 1:18:2])
            nc.vector.tensor_mul(apack[:, sl, :, 1], am1[:, sl], alo[:, sl, 1:18:2])
            nc.vector.tensor_mul(apack[:, sl, :, 2], am0[:, sl], ahi[:, sl, 1:18:2])
            nc.vector.tensor_mul(a3[:, sl], am1[:, sl], ahi[:, sl, 1:18:2])

        # ---------------- per-pc consumers ----------------
        def emit_dg(pc):
            """Diag tiles for one p-chunk: dg3[:, k, n, :] = a_n(q,k) * I."""
            dg = dgpool.tile([128, K2, 4, 128], BF16, tag="dg3",
                             name=f"dg3_{pc}")
            nc.vector.tensor_tensor(
                dg[:, :, 0:3, :],
                identb[:, None, None, :].broadcast_to([128, K2, 3, 128]),
                apack[:, pc, :, :, None].broadcast_to([128, K2, 3, 128]),
                ALU.mult)
            for k in range(K2):
                nc.scalar.mul(dg[:, k, 3, :], identb[:], a3[:, pc, k:k + 1])
            return dg

        x2_ap = x2_d[:]
        x2_view = bass.AP(tensor=x2_ap.tensor, offset=0,
                          ap=[[512, X2N - 2], [1, 1024]])

        def emit_mm(nt):
            for oc in range(2):
                psm = ps_mm.tile([128, 384], F32, tag="mm")
                for cc in range(CC):
                    nc.tensor.matmul(
                        psm[:], wdc[:, cc, oc * 128:(oc + 1) * 128],
                        patches[:, cc, nt * 384:(nt + 1) * 384],
                        start=(cc == 0), stop=(cc == CC - 1))
                if oc == 0:
                    nc.vector.tensor_copy(out_sb[:, oc, nt * 384:(nt + 1) * 384], psm[:])
                else:
                    nc.scalar.copy(out_sb[:, oc, nt * 384:(nt + 1) * 384], psm[:])

        # ---------------- emission schedule ----------------
        emit_om(0)
        emit_idx([0, 1, 2])
        emit_coef([0, 1, 2])
        emit_om(1)
        emit_om(2)
        dgs = {}
        dgs[0] = emit_dg(0)
        dgs[1] = emit_dg(1)
        emit_idx([3, 4, 5, 6, 7, 8])
        dgs[2] = emit_dg(2)
        emit_coef([3, 4, 5, 6, 7, 8])

        if stage <= 2:
            out_f = work.tile([128, PC, K2], F32, name="out_f")
            nc.vector.tensor_copy(out_f[:], slotf[:])
            nc.sync.dma_start(out=out_d[0][:, 0:PC * K2], in_=out_f[:])
            return nc

        for pc in range(PC):
            gt = gpool.tile([128, K2, 1024], BF16, tag="gt")
            nc.gpsimd.dma_gather(
                gt[:], x2_view, idxbuf[:, pc, :, :], P, P, 1024, elem_step=512,
                single_packet=False, queue_num=pc % 2)

            if stage <= 3:
                if pc == 0:
                    nc.gpsimd.dma_start(out=out_d[0][:, 0:1024], in_=gt[:, 0, :])
                continue

            dg = dgs.pop(pc) if pc in dgs else emit_dg(pc)

            # fused scale+transpose on PE: psum[c, q] = sum_n gt_n^T @ diag(a_n)
            for g0 in range(0, CC, 4):
                ng = min(4, CC - g0)
                pst = ps_tr.tile([128, 4, 128], F32, tag="tp")
                for j in range(ng):
                    cc = g0 + j
                    k, ch = cc // 2, cc % 2
                    for n in range(4):
                        nc.tensor.matmul(
                            pst[:, j, :],
                            gt[:, k, n * 256 + ch * 128: n * 256 + ch * 128 + 128],
                            dg[:, k, n, :],
                            start=(n == 0), stop=(n == 3))
                nc.scalar.copy(
                    patches[:, g0:g0 + ng, pc * 128:(pc + 1) * 128],
                    pst[:, 0:ng, :])

            if stage <= 4:
                if pc == 0:
                    nc.gpsimd.dma_start(out=out_d[0][:, 0:1152],
                                        in_=patches[:, 0, 0:1152])
                continue

            if pc % 3 == 2:
                emit_mm(pc // 3)

        if stage > 4:
            nc.sync.dma_start(out=out_d[0], in_=out_sb[:, 0, :])
            nc.scalar.dma_start(out=out_d[1], in_=out_sb[:, 1, :])

    return nc


def prep_core_inputs(x, w_offset, w_dcn, core):
    """Host-side layout prep for one core (layout/cast only, no math)."""
    b, h = core // 2, core % 2
    i0 = ROWS * h
    xb = x.astype(bf16)

    # xcyx: [2, 128, XROWS*50] padded rows i0-1 .. i0+24
    xc = np.zeros((2, 128, XROWS, 50), bf16)
    for r in range(XROWS):
        xr = i0 + r - 1
        if 0 <= xr < H:
            xc[0, :, r, 1:49] = xb[b, 0:128, xr, :]
            xc[1, :, r, 1:49] = xb[b, 128:256, xr, :]
    xc = xc.reshape(2, 128, XROWS * 50)

    # x2 dup-row: [X2N*512]
    xpad2 = np.zeros((51, 50, CIN), bf16)
    xpad2[1:49, 1:49] = np.transpose(xb[b], (1, 2, 0))
    x2 = np.concatenate([xpad2[0:50], xpad2[1:51]], axis=-1).reshape(2500, 512)
    x2 = np.concatenate([x2, np.zeros((X2N - 2500, 512), bf16)], axis=0)

    # base tables, y/x interleaved (minus 64 folded from the floor trick)
    pp = np.arange(128)
    pcs = np.arange(PC)
    p = pcs[None, :] * 128 + pp[:, None]          # [128, PC]
    i = i0 + p // W
    j = p % W
    ki = (np.arange(K2) // K)
    kj = (np.arange(K2) % K)
    baseyx = np.empty((128, PC, 18), np.float32)
    baseyx[:, :, 0::2] = (i[:, :, None] - 1 + ki[None, None, :] - 64).astype(np.float32)
    baseyx[:, :, 1::2] = (j[:, :, None] - 1 + kj[None, None, :] - 64).astype(np.float32)

    # weights
    wo = np.zeros((128, CC, 27), bf16)
    wd = np.zeros((128, CC, 256), bf16)
    w_off_b = w_offset.astype(bf16)
    w_dcn_b = w_dcn.astype(bf16)
    for k in range(K2):
        kii, kjj = k // K, k % K
        for ch in range(2):
            wo[:, k * 2 + ch, :] = w_off_b[:, ch * 128:(ch + 1) * 128, kii, kjj].T
            wd[:, k * 2 + ch, :] = w_dcn_b[:, ch * 128:(ch + 1) * 128, kii, kjj].T

    return {
        "xcyx": xc,
        "x2": x2.reshape(-1),
        "baseyx": baseyx,
        "woff": wo,
        "wdcn": wd,
        "identf": np.eye(128, dtype=np.float32),
        "identb": np.eye(128, dtype=np.float32).astype(bf16),
    }


_CACHED = {}
TRACE = False          # set True (e.g. from test.py) to capture an NTFF profile
LAST = {}              # exec_time_ns / profile info from the last run


def kernel(x, w_offset, w_dcn):
    from concourse.bass_utils import run_bass_kernel_spmd

    x = np.asarray(x, np.float32)
    w_offset = np.asarray(w_offset, np.float32)
    w_dcn = np.asarray(w_dcn, np.float32)

    if "nc" not in _CACHED:
        nc = build_nc()
        nc.finalize()
        _CACHED["nc"] = nc
    nc = _CACHED["nc"]

    in_maps = [prep_core_inputs(x, w_offset, w_dcn, c) for c in range(NCORES)]
    kr = run_bass_kernel_spmd(nc, in_maps, list(range(NCORES)), trace=TRACE)
    res = kr.results
    LAST["exec_time_ns"] = kr.exec_time_ns
    LAST["results"] = kr

    out = np.empty((B, COUT, H, W), np.float32)
    for core in range(NCORES):
        b, h = core // 2, core % 2
        i0 = ROWS * h
        o = res[core]["out"]          # [2, 128, P]
        out[b, 0:128, i0:i0 + ROWS, :] = o[0].reshape(128, ROWS, W)
        out[b, 128:256, i0:i0 + ROWS, :] = o[1].reshape(128, ROWS, W)
    return out


# revision 10
# speedup vs baseline: 1.0945x; 1.0945x over previous
"""DeformableConvV2 (DCNv2) Trainium2 Bass kernel, v2.

Problem (hardcoded): x [4,256,48,48] f32, w_offset [27,256,3,3], w_dcn
[256,256,3,3]; stride 1, dil 1, same padding -> out [4,256,48,48] f32.

Strategy: 8 cores, each handles half a sample (24 output rows, p=1152
positions).  Per core:
  1. offset/mask conv on PE (bf16 operands, f32 PSUM), pipelined per
     384-position n-tile so index math starts early
  2. transpose om to [p, 27]; merged y/x fp32 index math on DVE over
     interleaved [p, 18] tiles; bilinear+mask coeffs packed to
     apack [p, k, 4] bf16
  3. int16 element indices -> dma_gather from a host-prepared "dup-row"
     NHWC bf16 image: one 2KB element = all 4 bilinear neighbors (256ch)
  4. per p-chunk: diag tiles built in bulk (one DVE tensor_tensor with
     broadcast APs for neighbors 0-2, ACT muls for neighbor 3), then
     fused scale+transpose on PE: psum[c, q] += gt_n^T @ diag(a_n)
  5. main contraction on PE from patches [(k,c), p]
"""
import numpy as np
import ml_dtypes
from contextlib import ExitStack

import concourse.bass as bass
import concourse.bacc as bacc
import concourse.mybir as mybir
from concourse.tile import TileContext

bf16 = ml_dtypes.bfloat16
F32 = mybir.dt.float32
BF16 = mybir.dt.bfloat16
I16 = mybir.dt.int16
I32 = mybir.dt.int32
ALU = mybir.AluOpType
ACTF = mybir.ActivationFunctionType

B, CIN, COUT, K, H, W = 4, 256, 256, 3, 48, 48
K2 = K * K
NCORES = 8
ROWS = H // 2              # output rows per core = 24
P = ROWS * W               # positions per core = 1152
PC = P // 128              # p-chunks per core = 9
CC = 2 * K2                # contraction chunks = 18  (k*2 + c_half)
XROWS = ROWS + 2           # padded x rows needed for om conv = 26
X2N = 50 * 50 + 8          # dup-row gather source rows (+slack)
NTAP = 3                   # main-matmul n-tiles of 384 positions


def build_nc(stage=99):
    """Build the single SPMD program (same for all 8 cores).

    stage: debug truncation level (99 = full kernel):
      2 = loads + om conv + index math (slotf out);  3 = + gathers;
      4 = + patches;  99 = full
    """
    nc = bacc.Bacc(num_swdge_queues=2)

    xc_d = nc.declare_dram_parameter("xcyx", [2, 128, XROWS * 50], BF16, isOutput=False)
    x2_d = nc.declare_dram_parameter("x2", [X2N * 512], BF16, isOutput=False)
    byx_d = nc.declare_dram_parameter("baseyx", [128, PC, 18], F32, isOutput=False)
    wo_d = nc.declare_dram_parameter("woff", [128, CC, 27], BF16, isOutput=False)
    wd_d = nc.declare_dram_parameter("wdcn", [128, CC, 256], BF16, isOutput=False)
    idf_d = nc.declare_dram_parameter("identf", [128, 128], F32, isOutput=False)
    idb_d = nc.declare_dram_parameter("identb", [128, 128], BF16, isOutput=False)
    out_d = nc.declare_dram_parameter("out", [2, 128, P], F32, isOutput=True)

    with TileContext(nc) as tc, ExitStack() as ctx:
        const = ctx.enter_context(tc.tile_pool(name="const", bufs=1))
        work = ctx.enter_context(tc.tile_pool(name="work", bufs=1))
        gpool = ctx.enter_context(tc.tile_pool(name="gpool", bufs=4))
        dgpool = ctx.enter_context(tc.tile_pool(name="dgpool", bufs=3))
        ps_om = ctx.enter_context(tc.tile_pool(name="ps_om", bufs=1, space="PSUM"))
        ps_tr = ctx.enter_context(tc.tile_pool(name="ps_tr", bufs=2, space="PSUM"))
        ps_mm = ctx.enter_context(tc.tile_pool(name="ps_mm", bufs=2, space="PSUM"))

        # ---------------- loads (om-conv critical ones first) ----------------
        wof = const.tile([128, CC, 27], BF16)
        nc.sync.dma_start(out=wof[:], in_=wo_d[:])
        xc = const.tile([128, 2, XROWS * 50], BF16)
        nc.sync.dma_start(out=xc[:], in_=xc_d.rearrange("a p f -> p a f"))
        identf = const.tile([128, 128], F32)
        nc.scalar.dma_start(out=identf[:], in_=idf_d[:])
        baseyx = const.tile([128, PC, 18], F32)
        nc.scalar.dma_start(out=baseyx[:], in_=byx_d[:])
        identb = const.tile([128, 128], BF16)
        nc.sync.dma_start(out=identb[:], in_=idb_d[:])
        wdc = const.tile([128, CC, 256], BF16)
        nc.scalar.dma_start(out=wdc[:], in_=wd_d[:])

        # persistent tiles
        om_sb = work.tile([27, P], F32)
        omt = work.tile([128, PC, 27], F32)
        msk = work.tile([128, PC, K2], F32)
        apack = work.tile([128, PC, K2, 4], BF16)
        slot16 = work.tile([128, PC, K2], I16)
        idxtmp = work.tile([128, 8, PC, K2], I16)
        idxbuf = work.tile([128, PC, K2, 8], I16)
        patches = work.tile([128, CC, P], BF16)
        out_sb = work.tile([128, 2, P], F32)

        shape18 = [128, PC, 18]

        def wt18(name, dt=F32):
            return work.tile(shape18, dt, name=name)

        def wt9(name, dt=F32):
            return work.tile([128, PC, K2], dt, name=name)

        t_ = wt18("t_")
        ti = wt18("ti", I32)
        tf = wt18("tf")
        fx = wt18("fx")
        fl = wt18("fl")
        wv = wt18("wv")
        v0 = wt18("v0")
        ov = wt18("ov")
        ulo = wt18("ulo")
        alo = wt18("alo")
        uhi = wt18("uhi")
        ahi = wt18("ahi")
        am0 = wt9("am0")
        am1 = wt9("am1")
        yc = wt9("yc")
        xcl = wt9("xcl")
        t1 = wt9("t1")
        slotf = wt9("slotf")

        # ---------------- om conv for one n-tile ----------------
        def emit_om(nt):
            ps = ps_om.tile([27, 384], F32, tag="om")
            first = True
            for k in range(K2):
                ki, kj = k // K, k % K
                r0 = nt * 8 + ki
                for ch in range(2):
                    rhs = xc[:, ch].rearrange("p (r j) -> p r j", r=XROWS)[
                        :, r0:r0 + 8, kj:kj + 48]
                    nc.tensor.matmul(
                        ps[:], wof[:, k * 2 + ch, :], rhs,
                        start=first, stop=(k == K2 - 1 and ch == 1))
                    first = False
            nc.scalar.copy(om_sb[:, nt * 384:(nt + 1) * 384], ps[:])

        # ---------------- omt + gather-index math for a group of pcs ----------
        def emit_idx(pcs):
            sl = slice(pcs[0], pcs[-1] + 1)
            for pc in pcs:
                pst = ps_tr.tile([128, 27], F32, tag="omt")
                nc.tensor.transpose(pst[:], om_sb[:, pc * 128:(pc + 1) * 128],
                                    identf[0:27, 0:27])
                nc.vector.tensor_copy(omt[:, pc, :], pst[:])

            d_all = omt[:, sl, 0:18]
            # floor via +64, int cast, and a fix for round-up casts:
            #   t = d + 64;  ti = cast_int(t);  fi = ti - (ti > t)  == floor(t)
            nc.vector.tensor_scalar_add(t_[:, sl], d_all, 64.0)
            nc.vector.tensor_copy(ti[:, sl], t_[:, sl])
            nc.vector.tensor_copy(tf[:, sl], ti[:, sl])
            nc.vector.tensor_tensor(fx[:, sl], tf[:, sl], t_[:, sl], ALU.is_gt)
            nc.vector.tensor_sub(fl[:, sl], tf[:, sl], fx[:, sl])
            nc.vector.tensor_sub(wv[:, sl], t_[:, sl], fl[:, sl])
            nc.vector.tensor_add(v0[:, sl], fl[:, sl], baseyx[:, sl])

            # int element index into the dup-row table:
            #   slot = (yc+1)*50 + (xc+1), yc in [-1,47], xc in [-1,48]
            nc.vector.tensor_scalar(yc[:, sl], v0[:, sl, 0:18:2], -1.0, 47.0,
                                    ALU.max, ALU.min)
            nc.vector.tensor_scalar(xcl[:, sl], v0[:, sl, 1:18:2], -1.0, 48.0,
                                    ALU.max, ALU.min)
            nc.vector.tensor_scalar(t1[:, sl], yc[:, sl], 50.0, 51.0,
                                    ALU.mult, ALU.add)
            nc.vector.tensor_add(slotf[:, sl], t1[:, sl], xcl[:, sl])
            nc.vector.tensor_copy(slot16[:, sl], slotf[:, sl])

            # idx fold: [128, pc, k] -> 16-partition wrap + hi shuffle + replicate
            for s0 in range(pcs[0], pcs[-1] + 1, 3):
                s3 = slice(s0, s0 + 3)
                for hi in range(8):
                    eng = nc.sync if hi % 2 == 0 else nc.scalar
                    eng.dma_start(out=idxtmp[0:16, hi, s3, :],
                                  in_=slot16[hi * 16:(hi + 1) * 16, s3, :])
                nc.vector.tensor_copy(
                    idxbuf[0:16, s3, :, :],
                    idxtmp[0:16, :, s3, :].rearrange("p a b c -> p b c a"))
                for r in range(7):   # 7 parallel replicas of partitions 0:16
                    eng = nc.sync if r % 2 == 0 else nc.scalar
                    eng.dma_start(out=idxbuf[(r + 1) * 16:(r + 2) * 16, s3],
                                  in_=idxbuf[0:16, s3])

            # mask (ACT)
            nc.scalar.activation(msk[:, sl], omt[:, sl, 18:27], ACTF.Sigmoid)

        def emit_coef(pcs):
            sl = slice(pcs[0], pcs[-1] + 1)
            # bilinear weights x validity, y and x interleaved:
            #   alo = (1-w) * [v0 in [0,47]];  ahi = w * [v0+1 in [0,47]]
            nc.vector.tensor_scalar(ov[:, sl], wv[:, sl], -1.0, 1.0,
                                    ALU.mult, ALU.add)
            nc.vector.scalar_tensor_tensor(ulo[:, sl], v0[:, sl], 47.5,
                                           ov[:, sl], ALU.is_le, ALU.mult)
            nc.vector.scalar_tensor_tensor(alo[:, sl], v0[:, sl], -0.5,
                                           ulo[:, sl], ALU.is_ge, ALU.mult)
            nc.vector.scalar_tensor_tensor(uhi[:, sl], v0[:, sl], 46.5,
                                           wv[:, sl], ALU.is_le, ALU.mult)
            nc.vector.scalar_tensor_tensor(ahi[:, sl], v0[:, sl], -1.5,
                                           uhi[:, sl], ALU.is_ge, ALU.mult)

            nc.vector.tensor_mul(am0[:, sl], alo[:, sl, 0:18:2], msk[:, sl])
            nc.vector.tensor_mul(am1[:, sl], ahi[:, sl, 0:18:2], msk[:, sl])
            nc.vector.tensor_mul(apack[:, sl, :, 0], am0[:, sl], alo[:, sl, 1:18:2])
            nc.vector.tensor_mul(apack[:, sl, :, 1], am1[:, sl], alo[:, sl, 1:18:2])
            nc.vector.tensor_mul(apack[:, sl, :, 2], am0[:, sl], ahi[:, sl, 1:18:2])
            nc.vector.tensor_mul(apack[:, sl, :, 3], am1[:, sl], ahi[:, sl, 1:18:2])

        # ---------------- per-pc consumers ----------------
        def emit_dg(pc):
            """Diag tiles for one p-chunk: dg3[:, k, n, :] = a_n(q,k) * I."""
            dg = dgpool.tile([128, K2, 4, 128], BF16, tag="dg3",
                             name=f"dg3_{pc}")
            nc.vector.tensor_tensor(
                dg[:, :, :, :],
                identb[:, None, None, :].broadcast_to([128, K2, 4, 128]),
                apack[:, pc, :, :, None].broadcast_to([128, K2, 4, 128]),
                ALU.mult)
            return dg

        x2_ap = x2_d[:]
        x2_view = bass.AP(tensor=x2_ap.tensor, offset=0,
                          ap=[[512, X2N - 2], [1, 1024]])

        def emit_mm(nt, q0=None, qw=384):
            q0 = nt * 384 if q0 is None else q0
            for oc in range(2):
                psm = ps_mm.tile([128, 384], F32, tag="mm")
                for cc in range(CC):
                    nc.tensor.matmul(
                        psm[:, 0:qw], wdc[:, cc, oc * 128:(oc + 1) * 128],
                        patches[:, cc, q0:q0 + qw],
                        start=(cc == 0), stop=(cc == CC - 1))
                if oc == 0:
                    nc.vector.tensor_copy(out_sb[:, oc, q0:q0 + qw], psm[:, 0:qw])
                else:
                    nc.scalar.copy(out_sb[:, oc, q0:q0 + qw], psm[:, 0:qw])

        # ---------------- emission schedule ----------------
        emit_om(0)
        emit_idx([0, 1, 2])
        emit_coef([0, 1, 2])
        emit_om(1)
        emit_om(2)
        dgs = {}
        dgs[0] = emit_dg(0)
        dgs[1] = emit_dg(1)
        emit_idx([3, 4, 5, 6, 7, 8])
        dgs[2] = emit_dg(2)
        emit_coef([3, 4, 5, 6, 7, 8])

        if stage <= 2:
            out_f = work.tile([128, PC, K2], F32, name="out_f")
            nc.vector.tensor_copy(out_f[:], slotf[:])
            nc.sync.dma_start(out=out_d[0][:, 0:PC * K2], in_=out_f[:])
            return nc

        for pc in range(PC):
            gt = gpool.tile([128, K2, 1024], BF16, tag="gt")
            nc.gpsimd.dma_gather(
                gt[:], x2_view, idxbuf[:, pc, :, :], P, P, 1024, elem_step=512,
                single_packet=False, queue_num=pc % 2)

            if stage <= 3:
                if pc == 0:
                    nc.gpsimd.dma_start(out=out_d[0][:, 0:1024], in_=gt[:, 0, :])
                continue

            dg = dgs.pop(pc) if pc in dgs else emit_dg(pc)

            # fused scale+transpose on PE: psum[c, q] = sum_n gt_n^T @ diag(a_n)
            for g0 in range(0, CC, 4):
                ng = min(4, CC - g0)
                pst = ps_tr.tile([128, 4, 128], F32, tag="tp")
                for j in range(ng):
                    cc = g0 + j
                    k, ch = cc // 2, cc % 2
                    for n in range(4):
                        nc.tensor.matmul(
                            pst[:, j, :],
                            gt[:, k, n * 256 + ch * 128: n * 256 + ch * 128 + 128],
                            dg[:, k, n, :],
                            start=(n == 0), stop=(n == 3))
                nc.scalar.copy(
                    patches[:, g0:g0 + ng, pc * 128:(pc + 1) * 128],
                    pst[:, 0:ng, :])

            if stage <= 4:
                if pc == 0:
                    nc.gpsimd.dma_start(out=out_d[0][:, 0:1152],
                                        in_=patches[:, 0, 0:1152])
                continue

            if pc >= 6:
                emit_mm(2, q0=pc * 128, qw=128)   # final tile per-pc: short tail
            elif pc % 3 == 2:
                emit_mm(pc // 3)

        if stage > 4:
            nc.sync.dma_start(out=out_d[0], in_=out_sb[:, 0, :])
            nc.scalar.dma_start(out=out_d[1], in_=out_sb[:, 1, :])

    return nc


def prep_core_inputs(x, w_offset, w_dcn, core):
    """Host-side layout prep for one core (layout/cast only, no math)."""
    b, h = core // 2, core % 2
    i0 = ROWS * h
    xb = x.astype(bf16)

    # xcyx: [2, 128, XROWS*50] padded rows i0-1 .. i0+24
    xc = np.zeros((2, 128, XROWS, 50), bf16)
    for r in range(XROWS):
        xr = i0 + r - 1
        if 0 <= xr < H:
            xc[0, :, r, 1:49] = xb[b, 0:128, xr, :]
            xc[1, :, r, 1:49] = xb[b, 128:256, xr, :]
    xc = xc.reshape(2, 128, XROWS * 50)

    # x2 dup-row: [X2N*512]
    xpad2 = np.zeros((51, 50, CIN), bf16)
    xpad2[1:49, 1:49] = np.transpose(xb[b], (1, 2, 0))
    x2 = np.concatenate([xpad2[0:50], xpad2[1:51]], axis=-1).reshape(2500, 512)
    x2 = np.concatenate([x2, np.zeros((X2N - 2500, 512), bf16)], axis=0)

    # base tables, y/x interleaved (minus 64 folded from the floor trick)
    pp = np.arange(128)
    pcs = np.arange(PC)
    p = pcs[None, :] * 128 + pp[:, None]          # [128, PC]
    i = i0 + p // W
    j = p % W
    ki = (np.arange(K2) // K)
    kj = (np.arange(K2) % K)
    baseyx = np.empty((128, PC, 18), np.float32)
    baseyx[:, :, 0::2] = (i[:, :, None] - 1 + ki[None, None, :] - 64).astype(np.float32)
    baseyx[:, :, 1::2] = (j[:, :, None] - 1 + kj[None, None, :] - 64).astype(np.float32)

    # weights
    wo = np.zeros((128, CC, 27), bf16)
    wd = np.zeros((128, CC, 256), bf16)
    w_off_b = w_offset.astype(bf16)
    w_dcn_b = w_dcn.astype(bf16)
    for k in range(K2):
        kii, kjj = k // K, k % K
        for ch in range(2):
            wo[:, k * 2 + ch, :] = w_off_b[:, ch * 128:(ch + 1) * 128, kii, kjj].T
            wd[:, k * 2 + ch, :] = w_dcn_b[:, ch * 128:(ch + 1) * 128, kii, kjj].T

    return {
        "xcyx": xc,
        "x2": x2.reshape(-1),
        "baseyx": baseyx,
        "woff": wo,
        "wdcn": wd,
        "identf": np.eye(128, dtype=np.float32),
        "identb": np.eye(128, dtype=np.float32).astype(bf16),
    }


_CACHED = {}
TRACE = False          # set True (e.g. from test.py) to capture an NTFF profile
LAST = {}              # exec_time_ns / profile info from the last run


def kernel(x, w_offset, w_dcn):
    from concourse.bass_utils import run_bass_kernel_spmd

    x = np.asarray(x, np.float32)
    w_offset = np.asarray(w_offset, np.float32)
    w_dcn = np.asarray(w_dcn, np.float32)

    if "nc" not in _CACHED:
        nc = build_nc()
        nc.finalize()
        _CACHED["nc"] = nc
    nc = _CACHED["nc"]

    in_maps = [prep_core_inputs(x, w_offset, w_dcn, c) for c in range(NCORES)]
    kr = run_bass_kernel_spmd(nc, in_maps, list(range(NCORES)), trace=TRACE)
    res = kr.results
    LAST["exec_time_ns"] = kr.exec_time_ns
    LAST["results"] = kr

    out = np.empty((B, COUT, H, W), np.float32)
    for core in range(NCORES):
        b, h = core // 2, core % 2
        i0 = ROWS * h
        o = res[core]["out"]          # [2, 128, P]
        out[b, 0:128, i0:i0 + ROWS, :] = o[0].reshape(128, ROWS, W)
        out[b, 128:256, i0:i0 + ROWS, :] = o[1].reshape(128, ROWS, W)
    return out
